# revision 21
# baseline (speedup 1.0000x reference)
"""ExpertLinear (dense MoE blend) Trainium2 kernel — expert-sharded.

y[b,o] = sum_k ew[b,k] * (x[b,:] @ W[k,o,:]) + sum_k ew[b,k] * bias[k,o]

Sharding: one expert per core (8 experts, 8 cores). Each core computes the
full-batch partial y_k[b,o] = ew[b,k] * (x[b,:] @ W[k,o,:]); the host sums
the 8 partials and adds the rank-E bias term ew @ bias (4M MACs, 0.01% of
the work, same order as the gather-sum itself). Per-core HBM traffic drops
from ~18.5 MB (data-parallel baseline, replicated W stream) to ~5 MB, and
m=128 matmuls (vs m=64) put PE busy at the chip-wide bf16 floor (~14 us),
which dominates the ~330 GB/s-shared per-core DMA stream.

Schedule notes (from trace analysis of earlier revisions):
  - x-tiles and the o-half-A weight tiles are interleaved per i-tile in
    ONE DRAM tensor (xwa), so a single in-order SP DMA stream feeds the
    PE both matmul operands chunk by chunk: every matmul's input waits
    collapse to one HWDGE-lane semaphore (walrus accepts at most ONE
    sync wait per instruction), with no absorber tricks.
  - Only 8 HWDGE sem lanes exist; lane reuse adds a queue-drain wait,
    which is fine for dep-free input DMAs but fatal for output DMAs
    (data wait + lane wait = 2). So: 6 input DMAs + ew4 + the critical
    last-bank store on HWDGE; the other stores ride SWDGE (Pool) lanes.
  - A few dep-free junk matmuls on a memset tile bridge the PE p-state
    ramp (1.2 -> 2.4 GHz after ~3 us of continuous busy; an idle gap
    resets it, which cost 3 us at half clock in rev 2).
  - PSUM phase A = o-cols 0:512 (i-tile-major, tracks the stream),
    phase B = o-cols 512:1024 (bank-major so banks finish staggered and
    evict/store pipeline through the tail). ACT evicts A, DVE evicts B;
    eviction fuses the ew scale (per-partition f32 scalar), so x and W
    carry a single bf16 rounding each.
  - bass_utils.get_walrus_args is patched to cap --max-sem-num: the
    NEFF epilogue zeroes every compiler-owned semaphore one EVENT_SEM
    at a time (~115 ns each, ~6.4 us for the default 253).
"""

import numpy as np

B, E, IN, OUT = 512, 8, 1024, 1024
NCORES = 8
P = 128
NIT = IN // P            # 8 i-tiles (contraction)
NBT = B // P             # 4 b-tiles (output partitions)
OH = OUT // 2            # 512-wide o-half (PSUM bank)
N_RAMP = 8               # fat junk matmuls that occupy PE until data lands
XW = NIT * (B + OH)      # xwa columns: per i-tile [x-tile | waA-tile]

_compiled = None


def _patch_drain_split():
    """The walrus build in this container rejects any instruction carrying
    more than one sync wait, including the kernel-tail Drain that
    TileContext emits with one wait per active semaphore. Split it into a
    sequence of single-wait drains (sequencer-FIFO keeps them ordered;
    the set of waits is identical)."""
    import concourse.tile as tile_mod

    if getattr(tile_mod.TileContext, "_drain_split_patched", False):
        return
    from concourse.tile_sem_assignment import N_PROCS
    from concourse.vector_clock import ScopedClock, VectorClock

    def _drain_and_barrier(self, tick_clock, wait_clock):
        gc = tick_clock.global_clock
        for p in range(N_PROCS):
            t = gc[p]
            if t <= 0:
                continue
            ticks = [0] * N_PROCS
            ticks[p] = t
            di = self.nc.sync.drain()
            wait_clock.add_sem_waits(
                di.ins, ScopedClock({None: VectorClock(ticks)})
            )
        self.nc.all_engine_barrier()
        assert self.sems is not None
        popped = self.nc._tile_sem_poison_stack.pop()
        assert popped is self._sem_poison
        self.nc.clear_and_free_semaphores(list(self.sems.allocated().values()))
        self.nc.all_engine_barrier()

    tile_mod.TileContext._drain_and_barrier = _drain_and_barrier
    tile_mod.TileContext._drain_split_patched = True


def _patch_walrus_sem_cap(cap=64):
    """Cap the compiler-owned semaphore file: the NEFF epilogue zeroes
    every sem individually (~115 ns each, engine-parallel), so the
    default 253-sem layout costs ~6 us of teardown inside the measured
    window."""
    import concourse.bass_utils as bu

    if getattr(bu, "_sem_cap_patched", False):
        return
    orig = bu.get_walrus_args

    def patched(*args, **kwargs):
        return [*orig(*args, **kwargs), f"--max-sem-num={cap}"]

    bu.get_walrus_args = patched
    bu._sem_cap_patched = True


def _build():
    import concourse.bass as bass
    import concourse.mybir as mybir
    import concourse.tile as tile

    _patch_drain_split()
    _patch_walrus_sem_cap()

    f32 = mybir.dt.float32
    bf16 = mybir.dt.bfloat16

    nc = bass.Bass()
    xwa_d = nc.dram_tensor("xwa", [P, XW], bf16, kind="ExternalInput")
    wb_d = nc.dram_tensor("wb", [P, NIT * OH], bf16, kind="ExternalInput")
    ew_d = nc.dram_tensor("ew4", [P, NBT], f32, kind="ExternalInput")
    # Partial sums leave as bf16 (one extra rounding on a partial, ~2e-3
    # relative on the final sum — tolerance is 2e-2): halves store
    # traffic and the tail-store transfer time.
    ya_d = nc.dram_tensor("ya", [P, NBT * OH], bf16, kind="ExternalOutput")
    yb_d = nc.dram_tensor("yb", [P, NBT * OH], bf16, kind="ExternalOutput")

    IW = B + OH  # 1024 xwa columns per i-tile

    with tile.TileContext(nc) as tc:
        with (
            tc.tile_pool(name="const", bufs=1) as const,
            tc.tile_pool(name="psum", bufs=1, space="PSUM") as psum,
        ):
            xw = const.tile([P, XW], bf16)
            wb = const.tile([P, NIT * OH], bf16)
            ew4 = const.tile([P, NBT], f32)
            jt = const.tile([1, OH], bf16)
            scr_a = const.tile([P, NBT], f32)
            scr_v = const.tile([P, NBT], f32)
            ya = const.tile([P, NBT * OH], bf16)
            yb = const.tile([P, NBT * OH], bf16)

            psa = [psum.tile([P, OH], f32, name=f"psa{t}") for t in range(NBT)]
            psb = [psum.tile([P, OH], f32, name=f"psb{t}") for t in range(NBT - 1)]
            # Bank 3 as two half-width tiles: per-tile deps let its first
            # half evict while the second still accumulates.
            psb3 = [
                psum.tile([P, OH // 2], f32, name=f"psb3{h}", tag=f"psa{h}")
                for h in range(2)
            ]

            # In-stream on SP, in consumption order: the i-tile chunks of
            # [x | waA], then the wb halves. Finer chunks let the PE track
            # the stream (each chunk's +0.9us completion-sem latency hides
            # behind the previous chunk's matmuls). ew4 rides a SWDGE lane
            # so lane 8 stays free for the tail-critical yb3 store.
            for lo, hi in ((0, 1), (1, 2), (2, 4), (4, 6), (6, 8)):
                nc.sync.dma_start(
                    xw[:, lo * IW:hi * IW], xwa_d[:, lo * IW:hi * IW]
                )
            wh = NIT * OH // 2
            nc.sync.dma_start(wb[:, 0:wh], wb_d[:, 0:wh])
            nc.sync.dma_start(wb[:, wh:], wb_d[:, wh:])
            nc.gpsimd.dma_start(ew4[:], ew_d[:])

            # PE clock-ramp starter: the DVFS ramp to 2.4 GHz appears
            # time-gated (~9 us) from the FIRST PE activity, so start PE
            # at kernel entry with zero-wait 1-column matmuls on the
            # preamble const AP (memset + all-engine barrier precede the
            # tile body, so no sync is needed). Then fatter junk matmuls
            # on the memset tile fill PE until the first chunk lands.
            # psb0 is reset by its real start=True group later.
            cj = nc.const_aps.aps[(mybir.dt.bfloat16, 1.0)]
            for _ in range(8):
                nc.tensor.matmul(
                    psb[0][0:1, 0:1], cj, cj, start=True, stop=True,
                )
            nc.vector.memset(jt[:], 1.0)
            for _ in range(N_RAMP):
                nc.tensor.matmul(
                    psb[0][:], jt[0:1, 0:P], jt[0:1, 0:OH],
                    start=True, stop=True,
                )

            # Phase A: o-cols 0:512, i-tile-major (tracks the stream).
            for it in range(NIT):
                for bt in range(NBT):
                    nc.tensor.matmul(
                        psa[bt][:],
                        xw[:, it * IW + bt * P:it * IW + (bt + 1) * P],
                        xw[:, it * IW + B:(it + 1) * IW],
                        start=(it == 0), stop=(it == NIT - 1),
                    )

            # Phase B: o-cols 512:1024, bank-major so banks finish
            # staggered and the evict/store tail pipelines. The last bank
            # runs as two half-column groups so its first half evicts
            # while the second half still accumulates.
            for bt in range(NBT - 1):
                for it in range(NIT):
                    nc.tensor.matmul(
                        psb[bt][:],
                        xw[:, it * IW + bt * P:it * IW + (bt + 1) * P],
                        wb[:, it * OH:(it + 1) * OH],
                        start=(it == 0), stop=(it == NIT - 1),
                    )
            hw_ = OH // 2
            for half in range(2):
                for it in range(NIT):
                    nc.tensor.matmul(
                        psb3[half][:],
                        xw[:, it * IW + 3 * P:it * IW + 4 * P],
                        wb[:, it * OH + half * hw_:it * OH + (half + 1) * hw_],
                        start=(it == 0), stop=(it == NIT - 1),
                    )

            # ACT: warm-up observes the ew4 lane, then evicts phase A with
            # the fused ew scale; the store rides a SWDGE lane (far off
            # the critical path).
            nc.scalar.mul(scr_a[:], ew4[:], 1.0)
            for bt in range(NBT):
                nc.scalar.mul(
                    ya[:, bt * OH:(bt + 1) * OH], psa[bt][:],
                    ew4[:, bt:bt + 1],
                )
            nc.gpsimd.dma_start(ya_d[:], ya[:])

            # DVE: warm-up, then evict phase B per bank. Stores: banks
            # {0,1} and {2} on SWDGE; the tail-critical bank 3 store uses
            # the one spare HWDGE lane (descriptor pre-enqueued on ACT,
            # fires the instant DVE's eviction sem ticks).
            nc.vector.tensor_scalar(
                scr_v[:], ew4[:], 1.0, None, mybir.AluOpType.mult
            )
            for bt in range(NBT - 1):
                nc.vector.tensor_scalar(
                    yb[:, bt * OH:(bt + 1) * OH], psb[bt][:],
                    ew4[:, bt:bt + 1], None, mybir.AluOpType.mult,
                )
                if bt == 1:
                    nc.gpsimd.dma_start(yb_d[:, 0:2 * OH], yb[:, 0:2 * OH])
                elif bt == 2:
                    nc.gpsimd.dma_start(
                        yb_d[:, 2 * OH:3 * OH], yb[:, 2 * OH:3 * OH]
                    )
            for half in range(2):
                nc.vector.tensor_scalar(
                    yb[:, 3 * OH + half * hw_:3 * OH + (half + 1) * hw_],
                    psb3[half][:],
                    ew4[:, 3:4], None, mybir.AluOpType.mult,
                )
            nc.scalar.dma_start(
                yb_d[:, 3 * OH:4 * OH], yb[:, 3 * OH:4 * OH]
            )

    return nc


def _get_compiled():
    global _compiled
    if _compiled is None:
        _compiled = _build()
    return _compiled


_w_cache = None


def _make_in_maps(x, expert_weights, weight):
    global _w_cache
    import ml_dtypes

    bf16 = ml_dtypes.bfloat16
    IW = B + OH

    x = np.asarray(x, dtype=np.float32)
    ew = np.asarray(expert_weights, dtype=np.float32)
    # xt[it][p, b] = x[b, it*128 + p]
    xt = x.T.reshape(NIT, P, B).astype(bf16)

    if _w_cache is None or _w_cache[0] is not weight:
        w = np.asarray(weight, dtype=np.float32)
        xwas, wbs = [], []
        for k in range(E):
            # wt[it][p, o] = W[k, o, it*128 + p]
            wt = w[k].T.reshape(NIT, P, OUT).astype(bf16)
            xwa = np.empty((P, XW), dtype=bf16)
            for it in range(NIT):
                xwa[:, it * IW + B:(it + 1) * IW] = wt[it, :, 0:OH]
            xwas.append(xwa)
            wbs.append(np.ascontiguousarray(
                wt[:, :, OH:OUT].transpose(1, 0, 2).reshape(P, NIT * OH)))
        _w_cache = (weight, xwas, wbs)
    xwas, wbs = _w_cache[1], _w_cache[2]
    # x changes per call: refresh the x columns of each core's xwa image.
    for xwa in xwas:
        for it in range(NIT):
            xwa[:, it * IW:it * IW + B] = xt[it]

    in_maps = []
    for c in range(NCORES):
        ew4 = np.ascontiguousarray(ew[:, c].reshape(NBT, P).T)  # [128, 4]
        in_maps.append({"xwa": xwas[c], "wb": wbs[c], "ew4": ew4})
    return in_maps


def kernel(x, expert_weights, weight, bias, _trace=False):
    from concourse.bass_utils import run_bass_kernel_spmd

    nc = _get_compiled()
    in_maps = _make_in_maps(x, expert_weights, weight)
    res = run_bass_kernel_spmd(
        nc, in_maps, core_ids=list(range(NCORES)), trace=_trace
    )
    # y[bt*128+p, oh*512+o] = y{a,b}[p, bt*512+o]; sum partials over cores.
    y = np.zeros((B, OUT), dtype=np.float32)
    for r in res.results:
        ya = np.asarray(r["ya"], dtype=np.float32)
        yb = np.asarray(r["yb"], dtype=np.float32)
        y[:, 0:OH] += ya.reshape(P, NBT, OH).transpose(1, 0, 2).reshape(B, OH)
        y[:, OH:OUT] += yb.reshape(P, NBT, OH).transpose(1, 0, 2).reshape(B, OH)
    # Rank-E bias term (B*E*OUT = 4M MACs, host-side like the gather-sum).
    y += np.asarray(expert_weights, dtype=np.float32) @ np.asarray(
        bias, dtype=np.float32
    )
    if _trace:
        return y, res
    return y


# revision 23
# speedup vs baseline: 1.2137x; 1.2137x over previous
"""ExpertLinear (dense MoE blend) Trainium2 kernel — expert-sharded.

y[b,o] = sum_k ew[b,k] * (x[b,:] @ W[k,o,:]) + sum_k ew[b,k] * bias[k,o]

Sharding: one expert per core (8 experts, 8 cores). Each core computes the
full-batch partial y_k[b,o] = ew[b,k] * (x[b,:] @ W[k,o,:]); the host sums
the 8 partials and adds the rank-E bias term ew @ bias (4M MACs, 0.01% of
the work, same order as the gather-sum itself). Per-core HBM traffic drops
from ~18.5 MB (data-parallel baseline, replicated W stream) to ~5 MB, and
m=128 matmuls (vs m=64) put PE busy at the chip-wide bf16 floor (~14 us),
which dominates the ~330 GB/s-shared per-core DMA stream.

Schedule notes (from trace analysis of earlier revisions):
  - x-tiles and the o-half-A weight tiles are interleaved per i-tile in
    ONE DRAM tensor (xwa), so a single in-order SP DMA stream feeds the
    PE both matmul operands chunk by chunk: every matmul's input waits
    collapse to one HWDGE-lane semaphore (walrus accepts at most ONE
    sync wait per instruction), with no absorber tricks.
  - Only 8 HWDGE sem lanes exist; lane reuse adds a queue-drain wait,
    which is fine for dep-free input DMAs but fatal for output DMAs
    (data wait + lane wait = 2). So: 6 input DMAs + ew4 + the critical
    last-bank store on HWDGE; the other stores ride SWDGE (Pool) lanes.
  - A few dep-free junk matmuls on a memset tile bridge the PE p-state
    ramp (1.2 -> 2.4 GHz after ~3 us of continuous busy; an idle gap
    resets it, which cost 3 us at half clock in rev 2).
  - PSUM phase A = o-cols 0:512 (i-tile-major, tracks the stream),
    phase B = o-cols 512:1024 (bank-major so banks finish staggered and
    evict/store pipeline through the tail). ACT evicts A, DVE evicts B;
    eviction fuses the ew scale (per-partition f32 scalar), so x and W
    carry a single bf16 rounding each.
  - bass_utils.get_walrus_args is patched to cap --max-sem-num: the
    NEFF epilogue zeroes every compiler-owned semaphore one EVENT_SEM
    at a time (~115 ns each, ~6.4 us for the default 253).
"""

import numpy as np

B, E, IN, OUT = 512, 8, 1024, 1024
NCORES = 8
P = 128
NIT = IN // P            # 8 i-tiles (contraction)
NBT = B // P             # 4 b-tiles (output partitions)
OH = OUT // 2            # 512-wide o-half (PSUM bank)
N_RAMP = 8               # fat junk matmuls that occupy PE until data lands
XW = NIT * (B + OH)      # xwa columns: per i-tile [x-tile | waA-tile]

_compiled = None


def _patch_drain_split():
    """The walrus build in this container rejects any instruction carrying
    more than one sync wait, including the kernel-tail Drain that
    TileContext emits with one wait per active semaphore. Split it into a
    sequence of single-wait drains (sequencer-FIFO keeps them ordered;
    the set of waits is identical)."""
    import concourse.tile as tile_mod

    if getattr(tile_mod.TileContext, "_drain_split_patched", False):
        return
    from concourse.tile_sem_assignment import N_PROCS
    from concourse.vector_clock import ScopedClock, VectorClock

    def _drain_and_barrier(self, tick_clock, wait_clock):
        gc = tick_clock.global_clock
        for p in range(N_PROCS):
            t = gc[p]
            if t <= 0:
                continue
            ticks = [0] * N_PROCS
            ticks[p] = t
            di = self.nc.sync.drain()
            wait_clock.add_sem_waits(
                di.ins, ScopedClock({None: VectorClock(ticks)})
            )
        self.nc.all_engine_barrier()
        assert self.sems is not None
        popped = self.nc._tile_sem_poison_stack.pop()
        assert popped is self._sem_poison
        self.nc.clear_and_free_semaphores(list(self.sems.allocated().values()))
        self.nc.all_engine_barrier()

    tile_mod.TileContext._drain_and_barrier = _drain_and_barrier
    tile_mod.TileContext._drain_split_patched = True


def _patch_walrus_sem_cap(cap=64):
    """Cap the compiler-owned semaphore file: the NEFF epilogue zeroes
    every sem individually (~115 ns each, engine-parallel), so the
    default 253-sem layout costs ~6 us of teardown inside the measured
    window."""
    import concourse.bass_utils as bu

    if getattr(bu, "_sem_cap_patched", False):
        return
    orig = bu.get_walrus_args

    def patched(*args, **kwargs):
        return [*orig(*args, **kwargs), f"--max-sem-num={cap}"]

    bu.get_walrus_args = patched
    bu._sem_cap_patched = True


def _build():
    import concourse.bass as bass
    import concourse.mybir as mybir
    import concourse.tile as tile

    _patch_drain_split()
    _patch_walrus_sem_cap()

    f32 = mybir.dt.float32
    bf16 = mybir.dt.bfloat16

    nc = bass.Bass()
    xwa_d = nc.dram_tensor("xwa", [P, XW], bf16, kind="ExternalInput")
    wb_d = nc.dram_tensor("wb", [P, NIT * OH], bf16, kind="ExternalInput")
    ew_d = nc.dram_tensor("ew4", [P, NBT], f32, kind="ExternalInput")
    # Partial sums leave as bf16 (one extra rounding on a partial, ~2e-3
    # relative on the final sum — tolerance is 2e-2): halves store
    # traffic and the tail-store transfer time.
    ya_d = nc.dram_tensor("ya", [P, NBT * OH], bf16, kind="ExternalOutput")
    yb_d = nc.dram_tensor("yb", [P, NBT * OH], bf16, kind="ExternalOutput")

    IW = B + OH  # 1024 xwa columns per i-tile

    with tile.TileContext(nc) as tc:
        with (
            tc.tile_pool(name="const", bufs=1) as const,
            tc.tile_pool(name="psum", bufs=1, space="PSUM") as psum,
        ):
            xw = const.tile([P, XW], bf16)
            wb = const.tile([P, NIT * OH], bf16)
            ew4 = const.tile([P, NBT], f32)
            jt = const.tile([1, OH], bf16)
            scr_a = const.tile([P, NBT], f32)
            scr_v = const.tile([P, NBT], f32)
            ya = const.tile([P, NBT * OH], bf16)
            yb = const.tile([P, NBT * OH], bf16)

            psa = [psum.tile([P, OH], f32, name=f"psa{t}") for t in range(NBT)]
            psb = [psum.tile([P, OH], f32, name=f"psb{t}") for t in range(NBT)]

            # In-stream on SP, in consumption order: the i-tile chunks of
            # [x | waA], then the wb halves. Finer chunks let the PE track
            # the stream (each chunk's +0.9us completion-sem latency hides
            # behind the previous chunk's matmuls). ew4 rides a SWDGE lane
            # so lane 8 stays free for the tail-critical yb3 store.
            for lo, hi in ((0, 1), (1, 2), (2, 4), (4, 6), (6, 8)):
                nc.sync.dma_start(
                    xw[:, lo * IW:hi * IW], xwa_d[:, lo * IW:hi * IW]
                )
            wh = NIT * OH // 2
            nc.sync.dma_start(wb[:, 0:wh], wb_d[:, 0:wh])
            nc.sync.dma_start(wb[:, wh:], wb_d[:, wh:])
            nc.gpsimd.dma_start(ew4[:], ew_d[:])

            # PE clock-ramp starter: the DVFS ramp to 2.4 GHz appears
            # time-gated (~9 us) from the FIRST PE activity, so start PE
            # at kernel entry with zero-wait 1-column matmuls on the
            # preamble const AP (memset + all-engine barrier precede the
            # tile body, so no sync is needed). Then fatter junk matmuls
            # on the memset tile fill PE until the first chunk lands.
            # psb0 is reset by its real start=True group later.
            cj = nc.const_aps.aps[(mybir.dt.bfloat16, 1.0)]
            for _ in range(8):
                nc.tensor.matmul(
                    psb[0][0:1, 0:1], cj, cj, start=True, stop=True,
                )
            nc.vector.memset(jt[:], 1.0)
            for _ in range(N_RAMP):
                nc.tensor.matmul(
                    psb[0][:], jt[0:1, 0:P], jt[0:1, 0:OH],
                    start=True, stop=True,
                )

            # Phase A: o-cols 0:512, i-tile-major (tracks the stream).
            for it in range(NIT):
                for bt in range(NBT):
                    nc.tensor.matmul(
                        psa[bt][:],
                        xw[:, it * IW + bt * P:it * IW + (bt + 1) * P],
                        xw[:, it * IW + B:(it + 1) * IW],
                        start=(it == 0), stop=(it == NIT - 1),
                    )

            # Phase B: o-cols 512:1024, bank-major so banks finish
            # staggered and the evict/store tail pipelines. The last bank
            # runs as two half-column groups so its first half evicts
            # while the second half still accumulates.
            for bt in range(NBT):
                for it in range(NIT):
                    nc.tensor.matmul(
                        psb[bt][:],
                        xw[:, it * IW + bt * P:it * IW + (bt + 1) * P],
                        wb[:, it * OH:(it + 1) * OH],
                        start=(it == 0), stop=(it == NIT - 1),
                    )

            # ACT: warm-up observes the ew4 lane, then evicts phase A with
            # the fused ew scale; the store rides a SWDGE lane (far off
            # the critical path).
            nc.scalar.mul(scr_a[:], ew4[:], 1.0)
            for bt in range(NBT):
                nc.scalar.mul(
                    ya[:, bt * OH:(bt + 1) * OH], psa[bt][:],
                    ew4[:, bt:bt + 1],
                )
            nc.gpsimd.dma_start(ya_d[:], ya[:])

            # DVE: warm-up, then evict phase B per bank. Stores: banks
            # {0,1} and {2} on SWDGE; the tail-critical bank 3 store uses
            # the one spare HWDGE lane (descriptor pre-enqueued on ACT,
            # fires the instant DVE's eviction sem ticks).
            nc.vector.tensor_scalar(
                scr_v[:], ew4[:], 1.0, None, mybir.AluOpType.mult
            )
            for bt in range(NBT):
                nc.vector.tensor_scalar(
                    yb[:, bt * OH:(bt + 1) * OH], psb[bt][:],
                    ew4[:, bt:bt + 1], None, mybir.AluOpType.mult,
                )
                if bt == 1:
                    nc.gpsimd.dma_start(yb_d[:, 0:2 * OH], yb[:, 0:2 * OH])
                elif bt == 2:
                    nc.gpsimd.dma_start(
                        yb_d[:, 2 * OH:3 * OH], yb[:, 2 * OH:3 * OH]
                    )
                elif bt == 3:
                    nc.scalar.dma_start(
                        yb_d[:, 3 * OH:4 * OH], yb[:, 3 * OH:4 * OH]
                    )

    return nc


def _get_compiled():
    global _compiled
    if _compiled is None:
        _compiled = _build()
    return _compiled


_w_cache = None


def _make_in_maps(x, expert_weights, weight):
    global _w_cache
    import ml_dtypes

    bf16 = ml_dtypes.bfloat16
    IW = B + OH

    x = np.asarray(x, dtype=np.float32)
    ew = np.asarray(expert_weights, dtype=np.float32)
    # xt[it][p, b] = x[b, it*128 + p]
    xt = x.T.reshape(NIT, P, B).astype(bf16)

    if _w_cache is None or _w_cache[0] is not weight:
        w = np.asarray(weight, dtype=np.float32)
        xwas, wbs = [], []
        for k in range(E):
            # wt[it][p, o] = W[k, o, it*128 + p]
            wt = w[k].T.reshape(NIT, P, OUT).astype(bf16)
            xwa = np.empty((P, XW), dtype=bf16)
            for it in range(NIT):
                xwa[:, it * IW + B:(it + 1) * IW] = wt[it, :, 0:OH]
            xwas.append(xwa)
            wbs.append(np.ascontiguousarray(
                wt[:, :, OH:OUT].transpose(1, 0, 2).reshape(P, NIT * OH)))
        _w_cache = (weight, xwas, wbs)
    xwas, wbs = _w_cache[1], _w_cache[2]
    # x changes per call: refresh the x columns of each core's xwa image.
    for xwa in xwas:
        for it in range(NIT):
            xwa[:, it * IW:it * IW + B] = xt[it]

    in_maps = []
    for c in range(NCORES):
        ew4 = np.ascontiguousarray(ew[:, c].reshape(NBT, P).T)  # [128, 4]
        in_maps.append({"xwa": xwas[c], "wb": wbs[c], "ew4": ew4})
    return in_maps


def kernel(x, expert_weights, weight, bias, _trace=False):
    from concourse.bass_utils import run_bass_kernel_spmd

    nc = _get_compiled()
    in_maps = _make_in_maps(x, expert_weights, weight)
    res = run_bass_kernel_spmd(
        nc, in_maps, core_ids=list(range(NCORES)), trace=_trace
    )
    # y[bt*128+p, oh*512+o] = y{a,b}[p, bt*512+o]; sum partials over cores.
    y = np.zeros((B, OUT), dtype=np.float32)
    for r in res.results:
        ya = np.asarray(r["ya"], dtype=np.float32)
        yb = np.asarray(r["yb"], dtype=np.float32)
        y[:, 0:OH] += ya.reshape(P, NBT, OH).transpose(1, 0, 2).reshape(B, OH)
        y[:, OH:OUT] += yb.reshape(P, NBT, OH).transpose(1, 0, 2).reshape(B, OH)
    # Rank-E bias term (B*E*OUT = 4M MACs, host-side like the gather-sum).
    y += np.asarray(expert_weights, dtype=np.float32) @ np.asarray(
        bias, dtype=np.float32
    )
    if _trace:
        return y, res
    return y
